# revision 37
# baseline (speedup 1.0000x reference)
"""CRGCN multi-behavior GCN forward + BPR loss on 8 Trainium2 NeuronCores.

Strategy (edge/graph parallel, dst-range sharded):
  - Nodes padded to 150016 = 8 x 18752 rows; core c owns dst rows
    [c*18752, (c+1)*18752).
  - Per behavior, per layer: y = segment_sum(x[src] * enorm, dst) computed
    per-core over its edge shard via: banked int16 dma_gather of x rows,
    one-hot C built with tensor_scalar(is_equal (x) enorm) against an fp16
    iota, and PE matmuls accumulating y^T [64, 512-dst groups] in PSUM.
    Then x_new = (y @ W)  (W folded after the segment sum, since
    segsum((xW)[src]*e) == segsum(x[src]*e) @ W), written row-major via
    lhsT=y^T-slice matmuls. AllGather shards -> full table each layer.
  - Layer-2 epilogue: row normalize, residual add with layer_weight, then
    AllGather of the running `total`; BPR scoring is data-parallel (512
    batch rows per core) with int32 indirect row gathers.
  - Host does index-space preprocessing only (degree/enorm, sorting,
    sharding, padding); all tensor compute runs on device.

The SPMD program structure (chunk counts, PSUM windows, flags) is built
from the input data but made identical across cores by taking maxima /
unions over the 8 cores; per-core variation lives in tensor data only.
"""

import sys

if "/opt/trn_rl_repo" not in sys.path:
    sys.path.insert(0, "/opt/trn_rl_repo")

import numpy as np

import concourse.bass as bass
import concourse.tile as tile
from concourse import bacc, mybir
from concourse.bass_interp import get_hw_module
from concourse.bass_utils import run_bass_kernel_spmd

F32 = mybir.dt.float32
F16 = mybir.dt.float16
I16 = mybir.dt.int16
I32 = mybir.dt.int32

N_USERS = 50000
N_ITEMS = 100000
D = 64
N_BEH = 3
LAYERS = 2
BATCHSZ = 4096
N_NODES = N_USERS + N_ITEMS + 2  # 150002
NCORES = 8
SHARD = 18752
N_PAD = SHARD * NCORES  # 150016
GRP = 512
NGROUPS = (SHARD + GRP - 1) // GRP  # 37, last group 320 rows
LAST_GRP_ROWS = SHARD - (NGROUPS - 1) * GRP  # 320
BANKSZ = 32768
NBANKS = (N_PAD + BANKSZ - 1) // BANKSZ  # 5
BGROUPS = 4  # groups per gather batch
NBATCH = (NGROUPS + BGROUPS - 1) // BGROUPS  # 10
REG_WEIGHT = 1e-3
BPR_GAMMA = 1e-10
EMB_TILES = (SHARD + 127) // 128  # 147 (last tile 64 rows)

# debug bisection flags
SKIP_EMB = False
SKIP_LOSS = False
SKIP_LAYERS = False
SKIP_AG = False

MAX_GATHER = 1024  # SWDGE ring limit per dma_gather call (HW-probed;
# 2048 faults the runtime)
NQUEUES = 4
TW = 128  # fp16 table row width (64 data + 64 pad) -> 256B rows for gather
SUBAG = True  # sub-chunked AllGather overlapped with compute
CW = 128  # fixed one-hot window width (batched DVE is_equal build)

# Sub-AllGather parts: local shard rows [R_j, R_j + nr_j) are gathered as one
# contiguous block of the full table (part-major node layout, permutation
# applied host-side to src/loss indices).  Early parts are big (hidden under
# compute); the final part is tiny to minimise layer-boundary exposure.
if SUBAG:
    _PART_ROWS = [8192, 8192, 2048, 320]
    assert sum(_PART_ROWS) == SHARD
else:
    _PART_ROWS = [SHARD]
_PART_R0 = np.cumsum([0] + _PART_ROWS)[:-1].tolist()  # local row starts
_PART_P0 = np.cumsum([0] + [8 * n for n in _PART_ROWS])[:-1].tolist()
_NPARTS = len(_PART_ROWS)
# batch index at which each part's rows are complete -> fire its AllGather
_AG_AT = {}
for _j in range(_NPARTS):
    _AG_AT[(_PART_R0[_j] + _PART_ROWS[_j] - 1) // (BGROUPS * GRP)] = _j


def _part_of_row(r):
    for j in range(_NPARTS - 1, -1, -1):
        if r >= _PART_R0[j]:
            return j
    raise AssertionError


def _make_perm() -> np.ndarray:
    """node (padded, core-major) -> table row (part-major) permutation."""
    perm = np.empty(N_PAD, dtype=np.int64)
    for c in range(NCORES):
        for j in range(_NPARTS):
            r0, nr, p0 = _PART_R0[j], _PART_ROWS[j], _PART_P0[j]
            rows = np.arange(nr)
            perm[c * SHARD + r0 + rows] = p0 + c * nr + rows
    return perm


def _wrap_idx(vals: np.ndarray) -> np.ndarray:
    """int16 idx list (len % 16 == 0) -> [128, n/16] wrapped+replicated."""
    a = np.asarray(vals, dtype=np.int16).reshape(-1, 16).T  # [16, n/16]
    return np.tile(a, (8, 1))


def _row_chunks(rows: int):
    """Split `rows` into (k, nrows) pieces of 128 (last may be partial)."""
    out = []
    k = 0
    while rows > 0:
        out.append((k, min(128, rows)))
        rows -= 128
        k += 1
    return out


def preprocess(batch_data, edge_index, user_emb, item_emb, gcn_w, gcn_b, layer_weight):
    batch_data = np.asarray(batch_data)
    edge_index = np.asarray(edge_index)
    user_emb = np.asarray(user_emb, dtype=np.float32)
    item_emb = np.asarray(item_emb, dtype=np.float32)

    perm = _make_perm()
    x0_nat = np.zeros((N_PAD, TW), dtype=np.float16)
    x0_nat[: N_USERS + 1, :D] = user_emb.astype(np.float16)
    x0_nat[N_USERS + 1 : N_NODES, :D] = item_emb.astype(np.float16)
    x0_full = np.empty_like(x0_nat)
    x0_full[perm] = x0_nat  # part-major table layout
    x0_f32 = np.zeros((N_PAD, D), dtype=np.float32)
    x0_f32[: N_USERS + 1] = user_emb
    x0_f32[N_USERS + 1 : N_NODES] = item_emb

    # structure shared by all cores
    struct = {"beh": []}
    percore = [dict() for _ in range(NCORES)]

    for c in range(NCORES):
        percore[c]["x0_shard"] = np.ascontiguousarray(
            x0_f32[c * SHARD : (c + 1) * SHARD]
        )
        percore[c]["x0_shard16"] = np.ascontiguousarray(
            x0_nat[c * SHARD : (c + 1) * SHARD, :D]
        )
        # emb-loss masks [128, EMB_TILES]
        um = np.zeros((128, EMB_TILES), dtype=np.float32)
        im = np.zeros((128, EMB_TILES), dtype=np.float32)
        for t in range(EMB_TILES):
            rows = min(128, SHARD - t * 128)
            absrow = c * SHARD + t * 128 + np.arange(rows)
            um[:rows, t] = (absrow < N_USERS + 1).astype(np.float32)
            im[:rows, t] = (
                (absrow >= N_USERS + 1) & (absrow < N_NODES)
            ).astype(np.float32)
        percore[c]["umask"] = um
        percore[c]["imask"] = im

    iota = np.tile(np.arange(GRP, dtype=np.float16), (128, 1))

    for bi in range(N_BEH):
        src = np.asarray(edge_index[bi, 0], dtype=np.int64)
        dst = np.asarray(edge_index[bi, 1], dtype=np.int64)
        deg = np.bincount(dst, minlength=N_NODES).astype(np.float64)
        dinv = np.zeros(N_NODES, dtype=np.float64)
        nz = deg > 0
        dinv[nz] = 1.0 / np.sqrt(deg[nz])
        enorm_all = (dinv[src] * dinv[dst]).astype(np.float32)

        # per-core per-(bank, group) edge lists sorted by dst-local
        core_cells = []  # [c] -> dict (b, g) -> (es, dl_in_group, ee)
        for c in range(NCORES):
            base = c * SHARD
            sel = (dst >= base) & (dst < base + SHARD)
            es = perm[src[sel]]
            dl = (dst[sel] - base).astype(np.int64)
            ee = enorm_all[sel]
            g = dl >> 9
            b = es >> 15
            order = np.lexsort((dl, g, b))
            es, dl, ee, g, b = es[order], dl[order], ee[order], g[order], b[order]
            cells = {}
            key = b * NGROUPS + g
            bounds = np.flatnonzero(np.diff(key)) + 1
            starts = np.concatenate(([0], bounds))
            ends = np.concatenate((bounds, [len(key)]))
            for s, e in zip(starts, ends):
                if e > s:
                    cells[(int(b[s]), int(g[s]))] = (
                        es[s:e],
                        dl[s:e] & (GRP - 1),
                        ee[s:e],
                    )
            core_cells.append(cells)

        # fixed-width (CW) value-interval chunking, uniform across cores
        batches = []
        eidx_cols = [[] for _ in range(NCORES)]
        edl_cols = [[] for _ in range(NCORES)]
        een_cols = [[] for _ in range(NCORES)]
        empty = (
            np.zeros(0, np.int64),
            np.zeros(0, np.int64),
            np.zeros(0, np.float32),
        )
        for bt in range(NBATCH):
            glist = list(range(bt * BGROUPS, min((bt + 1) * BGROUPS, NGROUPS)))
            chunk_meta = []  # (b, g, wlo, wc)
            ces = [[] for _ in range(NCORES)]
            cdl = [[] for _ in range(NCORES)]
            cee = [[] for _ in range(NCORES)]
            call_n = []
            for b in range(NBANKS):
                nb0 = len(chunk_meta)
                for g in glist:
                    arrs = [
                        core_cells[c].get((b, g), empty) for c in range(NCORES)
                    ]
                    ptrs = [0] * NCORES
                    a0 = 0
                    while a0 < GRP:
                        rem = [
                            len(arrs[c][1]) - ptrs[c] for c in range(NCORES)
                        ]
                        if max(rem) == 0:
                            break
                        bend = min(a0 + CW, GRP)
                        for c in range(NCORES):
                            if rem[c] >= 128:
                                bend = min(bend, int(arrs[c][1][ptrs[c] + 127]))
                        if bend <= a0:
                            bend = a0 + 1
                        cnts = [
                            int(
                                np.searchsorted(arrs[c][1], bend, "left")
                                - ptrs[c]
                            )
                            if rem[c]
                            else 0
                            for c in range(NCORES)
                        ]
                        m = max(cnts)
                        if m > 0:
                            nchk = (m + 127) // 128
                            for s in range(nchk):
                                chunk_meta.append((b, g, a0, bend - a0))
                                for c in range(NCORES):
                                    lo = ptrs[c] + s * 128
                                    hi = min(ptrs[c] + cnts[c], lo + 128)
                                    if hi > lo:
                                        e_ = arrs[c][0][lo:hi]
                                        d_ = arrs[c][1][lo:hi] - a0
                                        w_ = arrs[c][2][lo:hi]
                                        pad = 128 - (hi - lo)
                                        ces[c].append(
                                            np.concatenate(
                                                (e_, np.full(pad, e_[-1]))
                                            )
                                        )
                                        cdl[c].append(
                                            np.concatenate(
                                                (d_, np.zeros(pad, np.int64))
                                            )
                                        )
                                        cee[c].append(
                                            np.concatenate(
                                                (w_, np.zeros(pad, np.float32))
                                            )
                                        )
                                    else:
                                        ces[c].append(
                                            np.full(128, b * BANKSZ, np.int64)
                                        )
                                        cdl[c].append(np.zeros(128, np.int64))
                                        cee[c].append(
                                            np.zeros(128, np.float32)
                                        )
                            for c in range(NCORES):
                                ptrs[c] += cnts[c]
                        a0 = bend
                call_n.append((len(chunk_meta) - nb0) * 128)
            nch = len(chunk_meta)
            # wrapped gather indices per bank call
            pos = 0
            for b in range(NBANKS):
                n = call_n[b]
                if n:
                    for c in range(NCORES):
                        es_b = np.concatenate(
                            ces[c][pos // 128 : (pos + n) // 128]
                        )
                        eidx_cols[c].append(_wrap_idx(es_b - b * BANKSZ))
                pos += n
            assert pos == nch * 128
            # metadata tiles: chunk kk -> columns of [128, nch]
            for c in range(NCORES):
                edl_cols[c].append(
                    np.stack(cdl[c], axis=0).T.astype(np.float16)
                )
                een_cols[c].append(
                    np.stack(cee[c], axis=0).T.astype(np.float16)
                )
            # stop flag on last chunk of each group within the batch
            last_seen = {}
            for i, (b, g, wlo, wc) in enumerate(chunk_meta):
                last_seen[g] = i
            chunks = [
                (wlo, wc, g, i == last_seen[g])
                for i, (b, g, wlo, wc) in enumerate(chunk_meta)
            ]
            batches.append(
                {"call_n": call_n, "chunks": chunks, "glist": glist}
            )

        tot_icol = sum(
            sum(n // 16 for n in b["call_n"]) for b in batches
        )
        tot_chk = sum(len(b["chunks"]) for b in batches)
        struct["beh"].append(
            {"batches": batches, "icol": tot_icol, "tch": tot_chk}
        )
        for c in range(NCORES):
            percore[c][f"eidx{bi}"] = np.concatenate(eidx_cols[c], axis=1)
            percore[c][f"edl{bi}"] = np.concatenate(edl_cols[c], axis=1)
            percore[c][f"een{bi}"] = np.concatenate(een_cols[c], axis=1)
            assert percore[c][f"eidx{bi}"].shape == (128, tot_icol)
            assert percore[c][f"edl{bi}"].shape == (128, tot_chk)

        # loss gather indices
        for c in range(NCORES):
            li = np.zeros((128, 12), dtype=np.int32)
            for k in range(4):
                rows = c * 512 + k * 128 + np.arange(128)
                li[:, k] = perm[batch_data[rows, bi, 0]]
                li[:, 4 + k] = perm[N_USERS + 1 + batch_data[rows, bi, 1]]
                li[:, 8 + k] = perm[N_USERS + 1 + batch_data[rows, bi, 2]]
            percore[c][f"lidx{bi}"] = li

    for c in range(NCORES):
        percore[c]["x0_full"] = x0_full
        percore[c]["iota"] = iota
        percore[c]["gw"] = np.asarray(gcn_w, dtype=np.float32)

    struct["has_bias"] = not np.all(np.asarray(gcn_b) == 0)
    struct["has_lw"] = not np.all(np.asarray(layer_weight) == 1)
    if struct["has_bias"]:
        for c in range(NCORES):
            percore[c]["gb"] = np.asarray(gcn_b, dtype=np.float32)
    if struct["has_lw"]:
        lw = np.asarray(layer_weight, dtype=np.float16)
        for c in range(NCORES):
            percore[c]["lw_shard"] = np.ascontiguousarray(
                lw[:, c * SHARD : (c + 1) * SHARD, :]
            )
    return struct, percore


def build(struct):
    nc = bacc.Bacc(
        "TRN2",
        target_bir_lowering=False,
        debug=False,
        num_devices=NCORES,
        num_swdge_queues=4,
    )
    d_x0full = nc.dram_tensor("x0_full", [N_PAD, TW], F16, kind="ExternalInput")
    d_x0sh = nc.dram_tensor("x0_shard", [SHARD, D], F32, kind="ExternalInput")
    d_x0sh16 = nc.dram_tensor("x0_shard16", [SHARD, D], F16, kind="ExternalInput")
    d_gw = nc.dram_tensor("gw", [N_BEH, LAYERS, D, D], F32, kind="ExternalInput")
    d_iota = nc.dram_tensor("iota", [128, GRP], F16, kind="ExternalInput")
    d_um = nc.dram_tensor("umask", [128, EMB_TILES], F32, kind="ExternalInput")
    d_im = nc.dram_tensor("imask", [128, EMB_TILES], F32, kind="ExternalInput")
    d_eidx, d_edl, d_een, d_lidx = [], [], [], []
    for bi in range(N_BEH):
        sb = struct["beh"][bi]
        d_eidx.append(
            nc.dram_tensor(f"eidx{bi}", [128, sb["icol"]], I16, kind="ExternalInput")
        )
        d_edl.append(
            nc.dram_tensor(f"edl{bi}", [128, sb["tch"]], F16, kind="ExternalInput")
        )
        d_een.append(
            nc.dram_tensor(f"een{bi}", [128, sb["tch"]], F16, kind="ExternalInput")
        )
        d_lidx.append(
            nc.dram_tensor(f"lidx{bi}", [128, 12], I32, kind="ExternalInput")
        )
    if struct["has_bias"]:
        d_gb = nc.dram_tensor("gb", [N_BEH, LAYERS, D], F32, kind="ExternalInput")
    if struct["has_lw"]:
        d_lw = nc.dram_tensor("lw_shard", [N_BEH, SHARD, D], F16, kind="ExternalInput")
    d_out = nc.dram_tensor("partials", [1, 8], F32, kind="ExternalOutput")

    # internal DRAM
    d_x1full = nc.dram_tensor("x1_full", [N_PAD, TW], F16, kind="Internal", addr_space="Shared")
    d_tfull = nc.dram_tensor("t_full", [N_PAD, TW], F16, kind="Internal", addr_space="Shared")
    d_x1p = [
        nc.dram_tensor(f"x1p{j}", [_PART_ROWS[j], TW], F16, kind="Internal")
        for j in range(_NPARTS)
    ]
    d_tp = [
        nc.dram_tensor(f"tp{j}", [_PART_ROWS[j], TW], F16, kind="Internal")
        for j in range(_NPARTS)
    ]

    with tile.TileContext(nc) as tc:
        with (
            tc.tile_pool(name="const", bufs=1) as cp,
            tc.tile_pool(name="meta", bufs=2) as mp,
            tc.tile_pool(name="msg", bufs=2) as gp,
            tc.tile_pool(name="chk", bufs=6) as kp,
            tc.tile_pool(name="epi", bufs=3) as ep,
            tc.tile_pool(name="acc", bufs=1) as ap_,
            tc.tile_pool(name="ps_y", bufs=5, space="PSUM") as ppy,
            tc.tile_pool(name="ps_w", bufs=2, space="PSUM") as ppw,
            tc.tile_pool(name="ps_f", bufs=1, space="PSUM") as ppf,
        ):
            iota_t = cp.tile([128, GRP], F16, name="iota_t")
            nc.sync.dma_start(out=iota_t[:], in_=d_iota[:])
            w16 = {}
            gw2 = d_gw[:].rearrange("b l p q -> (b l p) q")
            for bi in range(N_BEH):
                for li in range(LAYERS):
                    r0 = (bi * LAYERS + li) * D
                    wtmp = cp.tile([D, D], F32, name=f"wtmp{bi}{li}")
                    nc.sync.dma_start(out=wtmp[:], in_=gw2[r0 : r0 + D, :])
                    wt = cp.tile([D, D], F16, name=f"w16_{bi}{li}")
                    nc.vector.tensor_copy(out=wt[:], in_=wtmp[:])
                    w16[(bi, li)] = wt
            ones_t = cp.tile([128, 1], F32, name="ones_t")
            nc.vector.memset(ones_t[:], 1.0)
            zct = cp.tile([128, GRP], F16, name="zct")
            nc.vector.memset(zct[:], 0.0)
            gamma_t = cp.tile([128, 1], F32, name="gamma_t")
            nc.vector.memset(gamma_t[:], BPR_GAMMA)
            gb_bc = {}
            if struct["has_bias"]:
                # broadcast bias rows across partitions: ones[1,128]^T @ b[1,D]
                ones_row = cp.tile([1, 128], F32, name="ones_row")
                nc.vector.memset(ones_row[:], 1.0)
                for bi in range(N_BEH):
                    for li in range(LAYERS):
                        grow = cp.tile([1, D], F32, name=f"grow{bi}{li}")
                        gb2 = d_gb[:].rearrange("b l d -> (b l) d")
                        nc.sync.dma_start(
                            out=grow[:],
                            in_=gb2[bi * LAYERS + li : bi * LAYERS + li + 1, :],
                        )
                        pb = ppw.tile([128, D], F32, tag="pw", name=f"pb{bi}{li}")
                        nc.tensor.matmul(
                            out=pb[:], lhsT=ones_row[:], rhs=grow[:],
                            start=True, stop=True,
                        )
                        gbt = cp.tile([128, D], F32, name=f"gbbc{bi}{li}")
                        nc.vector.tensor_copy(out=gbt[:], in_=pb[:])
                        gb_bc[(bi, li)] = gbt

            fin_t = ap_.tile([128, 5], F32, name="fin_t")
            nc.vector.memset(fin_t[:], 0.0)

            if not SKIP_EMB:
                # ---- emb loss partial sums (independent; overlaps layer DMAs)
                um_t = cp.tile([128, EMB_TILES], F32, name="um_t")
                im_t = cp.tile([128, EMB_TILES], F32, name="im_t")
                nc.sync.dma_start(out=um_t[:], in_=d_um[:])
                nc.sync.dma_start(out=im_t[:], in_=d_im[:])
                acc_t = ap_.tile([128, EMB_TILES], F32, name="acc_t")
                nc.vector.memset(acc_t[:], 0.0)
                NB_EMB = 8
                for t0 in range(0, EMB_TILES, NB_EMB):
                    nt = min(NB_EMB, EMB_TILES - t0)
                    rows = min(128 * nt, SHARD - t0 * 128)
                    xin = ep.tile([128, NB_EMB, D], F32, tag="xin", name="xin")
                    full = rows // 128
                    if full:
                        nc.sync.dma_start(
                            out=xin[:, :full, :],
                            in_=d_x0sh[t0 * 128 : t0 * 128 + full * 128, :].rearrange(
                                "(k p) d -> p k d", p=128
                            ),
                        )
                    rem = rows - full * 128
                    if rem:
                        nc.sync.dma_start(
                            out=xin[:rem, full, :],
                            in_=d_x0sh[t0 * 128 + full * 128 : t0 * 128 + rows, :],
                        )
                    for k in range(nt):
                        pr = min(128, rows - k * 128)
                        junk = ep.tile([128, D], F16, tag="junk", name="junk")
                        nc.scalar.activation(
                            out=junk[:pr, :],
                            in_=xin[:pr, k, :],
                            func=mybir.ActivationFunctionType.Square,
                            accum_out=acc_t[:pr, t0 + k : t0 + k + 1],
                        )

                fin_t = ap_.tile([128, 5], F32, name="fin_t")
                tmp_m = ap_.tile([128, EMB_TILES], F32, name="tmp_m")
                nc.vector.tensor_tensor(
                    out=tmp_m[:], in0=acc_t[:], in1=um_t[:], op=mybir.AluOpType.mult
                )
                nc.vector.tensor_reduce(
                    out=fin_t[:, 3:4], in_=tmp_m[:], axis=mybir.AxisListType.X,
                    op=mybir.AluOpType.add,
                )
                nc.vector.tensor_tensor(
                    out=tmp_m[:], in0=acc_t[:], in1=im_t[:], op=mybir.AluOpType.mult
                )
                nc.vector.tensor_reduce(
                    out=fin_t[:, 4:5], in_=tmp_m[:], axis=mybir.AxisListType.X,
                    op=mybir.AluOpType.add,
                )

            # ---- behaviors
            for bi in range(N_BEH):
                sbh = struct["beh"][bi]
                if not SKIP_LAYERS:
                    for li in range(LAYERS):
                        src_tab = (
                            (d_x0full if bi == 0 else d_tfull)
                            if li == 0
                            else d_x1full
                        )
                        dst_parts = d_x1p if li == 0 else d_tp
                        icol0 = 0
                        tch0 = 0
                        for bt, binfo in enumerate(sbh["batches"]):
                            call_n = binfo["call_n"]
                            chunks = binfo["chunks"]
                            nch = len(chunks)
                            icol_b = sum(n // 16 for n in call_n)
                            idx_t = mp.tile([128, icol_b], I16, tag="idx", name="idx_t")
                            edl_t = mp.tile([128, nch], F16, tag="edl", name="edl_t")
                            een_t = mp.tile([128, nch], F16, tag="een", name="een_t")
                            nc.sync.dma_start(
                                out=idx_t[:], in_=d_eidx[bi][:, icol0 : icol0 + icol_b]
                            )
                            nc.sync.dma_start(
                                out=edl_t[:], in_=d_edl[bi][:, tch0 : tch0 + nch]
                            )
                            nc.sync.dma_start(
                                out=een_t[:], in_=d_een[bi][:, tch0 : tch0 + nch]
                            )
                            msg_t = gp.tile([128, nch, TW], F16, tag="msg", name="msg_t")
                            c0 = 0
                            i0 = 0
                            qn = 0
                            for b in range(NBANKS):
                                n = call_n[b]
                                hi_row = min(BANKSZ, N_PAD - b * BANKSZ)
                                while n > 0:
                                    # SWDGE descriptor ring caps one call at
                                    # ~1024 indices; larger calls deadlock.
                                    sub = min(n, MAX_GATHER)
                                    nc.gpsimd.dma_gather(
                                        out_ap=msg_t[:, c0 : c0 + sub // 128, :],
                                        in_ap=src_tab[
                                            b * BANKSZ : b * BANKSZ + hi_row, :
                                        ],
                                        idxs_ap=idx_t[:, i0 : i0 + sub // 16],
                                        num_idxs=sub,
                                        num_idxs_reg=sub,
                                        elem_size=TW,
                                        queue_num=qn % NQUEUES,
                                    )
                                    qn += 1
                                    c0 += sub // 128
                                    i0 += sub // 16
                                    n -= sub
                            # scale msg rows by enorm: one DVE op per batch
                            nc.vector.tensor_tensor(
                                out=msg_t[:, :, :D],
                                in0=msg_t[:, :, :D],
                                in1=een_t[:]
                                .unsqueeze(2)
                                .broadcast_to([128, nch, D]),
                                op=mybir.AluOpType.mult,
                            )
                            # batched one-hot build: one DVE op per batch
                            ct_all = gp.tile(
                                [128, nch, CW], F16, tag="ct", name="ct_all"
                            )
                            nc.vector.tensor_tensor(
                                out=ct_all[:],
                                in0=edl_t[:]
                                .unsqueeze(2)
                                .broadcast_to([128, nch, CW]),
                                in1=iota_t[:, :CW]
                                .unsqueeze(1)
                                .broadcast_to([128, nch, CW]),
                                op=mybir.AluOpType.is_equal,
                            )
                            # group psum tiles for this batch
                            glist = binfo["glist"]
                            gchunks = {g: 0 for g in glist}
                            for wlo, wc, g, sp in chunks:
                                gchunks[g] += 1
                            py = {
                                g: ppy.tile(
                                    [D, GRP], F32, tag="py", name=f"py{bi}{li}{g}"
                                )
                                for g in glist
                            }
                            for g in glist:
                                # PSUM init (zero matmul over the full group)
                                nc.tensor.matmul(
                                    out=py[g][:],
                                    lhsT=zct[:, :D],
                                    rhs=zct[:],
                                    start=True,
                                    stop=gchunks[g] == 0,
                                )
                            for kk, (wlo, wc, g, sp) in enumerate(chunks):
                                nc.tensor.matmul(
                                    out=py[g][:, wlo : wlo + wc],
                                    lhsT=msg_t[:, kk, :D],
                                    rhs=ct_all[:, kk, :wc],
                                    start=False,
                                    stop=sp,
                                )
                            # epilogues for this batch's groups
                            for g in glist:
                                g0 = g * GRP
                                rows = min(GRP, SHARD - g0)
                                pj = _part_of_row(g0)
                                po = g0 - _PART_R0[pj]
                                psh = dst_parts[pj]
                                y16 = ep.tile([D, GRP], F16, tag="y16", name="y16")
                                nc.scalar.copy(out=y16[:], in_=py[g][:])
                                xout = ep.tile([128, 4, TW], F16, tag="xout", name="xout")
                                for k, pr in _row_chunks(rows):
                                    pw = ppw.tile([128, D], F32, tag="pw", name="pw")
                                    nc.tensor.matmul(
                                        out=pw[:pr, :],
                                        lhsT=y16[:, k * 128 : k * 128 + pr],
                                        rhs=w16[(bi, li)][:],
                                        start=True,
                                        stop=True,
                                    )
                                    if struct["has_bias"]:
                                        nc.vector.tensor_tensor(
                                            out=pw[:pr, :],
                                            in0=pw[:pr, :],
                                            in1=gb_bc[(bi, li)][:pr, :],
                                            op=mybir.AluOpType.add,
                                        )
                                    if li == 0:
                                        nc.scalar.copy(
                                            out=xout[:pr, k, :D], in_=pw[:pr, :]
                                        )
                                    else:
                                        # normalize rows then residual
                                        ss = kp.tile([128, 1], F32, tag="ss", name="ss")
                                        junk2 = kp.tile(
                                            [128, D], F16, tag="junk2", name="junk2"
                                        )
                                        nc.scalar.activation(
                                            out=junk2[:pr, :],
                                            in_=pw[:pr, :],
                                            func=mybir.ActivationFunctionType.Square,
                                            accum_out=ss[:pr, :],
                                        )
                                        nrm = kp.tile([128, 1], F32, tag="nrm", name="nrm")
                                        nc.scalar.sqrt(out=nrm[:pr, :], in_=ss[:pr, :])
                                        nc.vector.tensor_scalar_max(
                                            out=nrm[:pr, :], in0=nrm[:pr, :], scalar1=1e-12
                                        )
                                        rn = kp.tile([128, 1], F32, tag="rn", name="rn")
                                        nc.vector.reciprocal(out=rn[:pr, :], in_=nrm[:pr, :])
                                        x2 = kp.tile([128, D], F16, tag="x2", name="x2")
                                        nc.vector.tensor_scalar_mul(
                                            out=x2[:pr, :], in0=pw[:pr, :], scalar1=rn[:pr, :]
                                        )
                                        if struct["has_lw"]:
                                            lwt = kp.tile(
                                                [128, D], F16, tag="lwt", name="lwt"
                                            )
                                            lw2 = d_lw[:].rearrange("b n d -> (b n) d")
                                            r0 = bi * SHARD + g0 + k * 128
                                            nc.sync.dma_start(
                                                out=lwt[:pr, :],
                                                in_=lw2[r0 : r0 + pr, :],
                                            )
                                            nc.vector.tensor_tensor(
                                                out=x2[:pr, :], in0=x2[:pr, :], in1=lwt[:pr, :],
                                                op=mybir.AluOpType.mult,
                                            )
                                        told = kp.tile([128, D], F16, tag="told", name="told")
                                        if bi == 0:
                                            nc.sync.dma_start(
                                                out=told[:pr, :],
                                                in_=d_x0sh16[
                                                    g0 + k * 128 : g0 + k * 128 + pr, :
                                                ],
                                            )
                                        else:
                                            nc.sync.dma_start(
                                                out=told[:pr, :],
                                                in_=d_tp[pj][
                                                    po + k * 128 : po + k * 128 + pr, :D
                                                ],
                                            )
                                        nc.vector.tensor_tensor(
                                            out=xout[:pr, k, :D], in0=x2[:pr, :], in1=told[:pr, :],
                                            op=mybir.AluOpType.add,
                                        )
                                # write out rows
                                full = rows // 128
                                if full:
                                    nc.sync.dma_start(
                                        out=psh[po : po + full * 128, :].rearrange(
                                            "(k p) d -> p k d", p=128
                                        ),
                                        in_=xout[:, :full, :],
                                    )
                                rem = rows - full * 128
                                if rem:
                                    nc.sync.dma_start(
                                        out=psh[po + full * 128 : po + rows, :],
                                        in_=xout[:rem, full, :],
                                    )
                            icol0 += icol_b
                            tch0 += nch
                            if not SKIP_AG and SUBAG and bt in _AG_AT:
                                # sub-chunked AllGather: ship the completed
                                # part while later batches still compute
                                # (part-major table layout keeps in/out
                                # contiguous; parts align with gather banks)
                                j = _AG_AT[bt]
                                ag_out = d_x1full if li == 0 else d_tfull
                                nc.gpsimd.collective_compute(
                                    "AllGather",
                                    mybir.AluOpType.bypass,
                                    replica_groups=[list(range(NCORES))],
                                    ins=[dst_parts[j][:]],
                                    outs=[
                                        ag_out[
                                            _PART_P0[j] : _PART_P0[j]
                                            + NCORES * _PART_ROWS[j],
                                            :,
                                        ]
                                    ],
                                )
                        if not SKIP_AG and not SUBAG:
                            # AllGather shard -> full
                            ag_out = d_x1full if li == 0 else d_tfull
                            nc.gpsimd.collective_compute(
                                "AllGather",
                                mybir.AluOpType.bypass,
                                replica_groups=[list(range(NCORES))],
                                ins=[dst_parts[0][:]],
                                outs=[ag_out[:]],
                            )

                if not SKIP_LOSS:
                    # ---- BPR loss partials for this behavior
                    lidx_t = cp.tile([128, 12], I32, name=f"lidx_t{bi}")
                    nc.sync.dma_start(out=lidx_t[:], in_=d_lidx[bi][:])
                    vb = ap_.tile([128, 4], F32, tag=f"vb{bi}", name=f"vb{bi}")
                    for k in range(4):
                        ut = kp.tile([128, TW], F16, tag="ut", name="ut")
                        pt = kp.tile([128, TW], F16, tag="pt", name="pt")
                        nt = kp.tile([128, TW], F16, tag="nt", name="nt")
                        for t, col in ((ut, k), (pt, 4 + k), (nt, 8 + k)):
                            nc.gpsimd.indirect_dma_start(
                                out=t[:],
                                out_offset=None,
                                in_=d_tfull[:],
                                in_offset=bass.IndirectOffsetOnAxis(
                                    ap=lidx_t[:, col : col + 1], axis=0
                                ),
                            )
                        tmp = kp.tile([128, D], F32, tag="tmp", name="tmp")
                        sp_ = kp.tile([128, 1], F32, tag="sp", name="sp")
                        sn_ = kp.tile([128, 1], F32, tag="sn", name="sn")
                        nc.vector.tensor_tensor(
                            out=tmp[:], in0=ut[:, :D], in1=pt[:, :D],
                            op=mybir.AluOpType.mult,
                        )
                        nc.vector.tensor_reduce(
                            out=sp_[:], in_=tmp[:], axis=mybir.AxisListType.X,
                            op=mybir.AluOpType.add,
                        )
                        nc.vector.tensor_tensor(
                            out=tmp[:], in0=ut[:, :D], in1=nt[:, :D],
                            op=mybir.AluOpType.mult,
                        )
                        nc.vector.tensor_reduce(
                            out=sn_[:], in_=tmp[:], axis=mybir.AxisListType.X,
                            op=mybir.AluOpType.add,
                        )
                        nc.vector.tensor_tensor(
                            out=sp_[:], in0=sp_[:], in1=sn_[:], op=mybir.AluOpType.subtract
                        )
                        sg = kp.tile([128, 1], F32, tag="sg", name="sg")
                        nc.scalar.activation(
                            out=sg[:], in_=sp_[:],
                            func=mybir.ActivationFunctionType.Sigmoid,
                        )
                        nc.scalar.activation(
                            out=vb[:, k : k + 1], in_=sg[:],
                            func=mybir.ActivationFunctionType.Ln,
                            bias=gamma_t[:],
                        )
                    nc.vector.tensor_reduce(
                        out=fin_t[:, bi : bi + 1], in_=vb[:], axis=mybir.AxisListType.X,
                        op=mybir.AluOpType.add,
                    )

            # ---- final partition reduce + output
            pf = ppf.tile([1, 5], F32, name="pf")
            nc.tensor.matmul(
                out=pf[:], lhsT=ones_t[:], rhs=fin_t[:], start=True, stop=True
            )
            fsb = ap_.tile([1, 8], F32, name="fsb")
            nc.vector.memset(fsb[:], 0.0)
            nc.vector.tensor_copy(out=fsb[:, :5], in_=pf[:])
            nc.sync.dma_start(out=d_out[:], in_=fsb[:])

    nc.compile()
    nc.m = get_hw_module(nc.m)
    return nc


def combine(partials):
    """partials: [NCORES, 8] -> scalar loss (float64 math, f32 out)."""
    p = np.asarray(partials, dtype=np.float64)
    bpr = -(p[:, 0:3].sum(axis=0) / BATCHSZ).sum()
    emb = (np.sqrt(p[:, 3].sum()) + np.sqrt(p[:, 4].sum())) / (N_ITEMS + 1)
    return np.float32(bpr + REG_WEIGHT * emb)


_BUILT = {}


def _run(inputs, trace=False, **kw):
    struct, percore = preprocess(**inputs)
    key = (
        tuple(s["icol"] for s in struct["beh"]),
        tuple(s["tch"] for s in struct["beh"]),
        struct["has_bias"],
        struct["has_lw"],
    )
    if key not in _BUILT:
        _BUILT[key] = build(struct)
    nc = _BUILT[key]
    res = run_bass_kernel_spmd(
        nc, percore, core_ids=list(range(NCORES)), trace=trace, **kw
    )
    parts = np.stack([res.results[c]["partials"][0] for c in range(NCORES)])
    return combine(parts), res


def kernel(**inputs) -> np.ndarray:
    out, _ = _run(inputs)
    return out

